# revision 1
# baseline (speedup 1.0000x reference)
"""Trainium2 Bass kernel for nn_BruteForceUpdater.

Reference computation:
    xs = x[:, 0, :]                       # [256, 128]
    U  = (xs @ W1.T) @ W2.T               # [256, 8256]
    fw_{i+1} = sigmoid(10*(fw_i + U_i - 0.5))   (serial over batch)
    pred_i = fw2_i @ relu(fw1_i @ x_i)    # fw1 = fw[:8192].reshape(64,128)

Distribution over 8 NeuronCores (no collectives; host sums partials):
  * NFW = 8256 = 64*128 (w1 part of the fast weights) + 64 (w2 part).
  * Core c owns fast-w1 row-tiles h in [8c, 8c+8) (1024 rows of W2) and
    every core also replicates the trailing 64 rows (the fast-w2 part),
    so each core processes a uniform slice of 1088 W2 rows.
  * The host hands each core ONE streaming tensor wcomb [16512, 1216]:
    cols 0:1088 = its W2 slice pre-transposed, cols 1088:1216 = the
    k-block-transposed updater W1 (each 128-row k-tile DMA carries both
    the GEMM weights and the W1 chunk for T1 = W1 @ xs.T).
  * Per core: T1.T tiles are produced on-device two k-tiles ahead of a
    129-tile K-accumulated GEMM into the U.T slice (float32r matmuls,
    psum sub-bank packed), then a 256-step sigmoid scan over the slice,
    then partial predictions sum_{h in core} w2_h*relu(H_h) as [1, 256].
    The host sums the 8 partials.

The streaming loop runs inside a tile_critical block with hand-rolled
semaphores: walrus allows only ~2 sync commands per LDWEIGHTS-matmul /
DMA pseudo-instruction, so each instruction here carries at most one
wait plus one increment, relying on the PE's in-order completion for
transitive coverage.
"""
import os
import sys

sys.path.insert(0, "/opt/trn_rl_repo")

import numpy as np
from contextlib import ExitStack

import concourse.bass as bass
import concourse.tile as tile
from concourse import mybir
from concourse.bass_utils import run_bass_kernel_spmd

F32 = mybir.dt.float32
F32R = mybir.dt.float32r
AF = mybir.ActivationFunctionType

IN = 128
HID = 64
NFW = IN * HID + HID          # 8256
B = 256
K2 = 2 * NFW                  # 16512
KT = K2 // 128                # 129 contraction tiles
NCORES = 8
MT_OWN = 8                    # full 128-row W2 tiles owned per core
NT = MT_OWN + 1               # + shared 64-row tile
MSL = MT_OWN * 128 + HID      # 1088 W2 rows handled per core
WC = MSL + 128                # streamed tile width (W2 cols + W1 chunk)
GAIN, SHIFT = 10.0, 0.5

NSLOT = 16                    # stream ring slots
LOOK = 12                     # DMA lookahead (<= NSLOT - 2)

NT1 = 4                       # t1 ring slots

_NC_CACHE = None


def _build_bass():
    nc = bass.Bass("TRN2", target_bir_lowering=False, debug=False)

    wc_d = nc.dram_tensor("wcomb", [K2, WC], F32R, kind="ExternalInput")
    cst_d = nc.dram_tensor("cst", [128, B + NT + MT_OWN + 1], F32,
                           kind="ExternalInput")
    pred_d = nc.dram_tensor("pred", [1, B], F32, kind="ExternalOutput")
    hs_d = nc.dram_tensor("hscratch", [MT_OWN, B], F32)
    dbg = bool(int(os.environ.get("KDBG", "0")))
    if dbg:
        u_dbg_d = nc.dram_tensor("u_dbg", [128, NT * B], F32,
                                 kind="ExternalOutput")
        fw_dbg_d = nc.dram_tensor("fw_dbg", [128, NT * B], F32,
                                  kind="ExternalOutput")
        t1_dbg_d = nc.dram_tensor("t1_dbg", [128, NT1 * B], F32,
                                  kind="ExternalOutput")
        hsb_dbg_d = nc.dram_tensor("hsb_dbg", [MT_OWN, B], F32,
                                   kind="ExternalOutput")
        w2t_dbg_d = nc.dram_tensor("w2t_dbg", [MT_OWN, B], F32,
                                   kind="ExternalOutput")
        hflat_dbg_d = nc.dram_tensor("hflat_dbg", [1, MT_OWN * B], F32,
                                     kind="ExternalOutput")

    with tile.TileContext(nc) as tc:
        with ExitStack() as ctx:
            const_pool = ctx.enter_context(tc.tile_pool(name="const", bufs=1))
            stream_pool = ctx.enter_context(tc.tile_pool(name="wcs", bufs=1))
            big_pool = ctx.enter_context(tc.tile_pool(name="big", bufs=1))

            cst = const_pool.tile([128, B + NT + MT_OWN + 1], F32)
            xst = cst[:, 0:B]
            fw0_t = cst[:, B:B + NT]
            sel_t = cst[0:HID, B + NT:B + NT + MT_OWN]
            ones_t = cst[:, B + NT + MT_OWN:B + NT + MT_OWN + 1]
            bias_t = const_pool.tile([128, 1], F32)
            zeros64_f = const_pool.tile([128, 128], F32)
            zeros64_r = const_pool.tile([128, 128], F32R)
            xst_r = const_pool.tile([128, B], F32R)

            wbuf = stream_pool.tile([128, NSLOT * WC], F32R)   # stream ring
            t1r = big_pool.tile([128, NT1 * B], F32R)          # T1.T ring
            u_sb = big_pool.tile([128, NT * B], F32)           # U.T slice
            fw_sb = big_pool.tile([128, NT * B], F32)          # fw history
            t_big = big_pool.tile([128, 2 * NT], F32)
            prod_big = big_pool.tile([128, MT_OWN * B], F32)
            w2t_sb = big_pool.tile([MT_OWN, B], F32)
            h_flat = big_pool.tile([1, MT_OWN * B], F32)
            h_sb = big_pool.tile([MT_OWN, B], F32)
            r_sb = big_pool.tile([MT_OWN, B], F32)
            p_sb = big_pool.tile([MT_OWN, B], F32)
            pred_sb = big_pool.tile([1, B], F32)

            u_r = u_sb[:].rearrange("p (m i) -> p m i", m=NT)
            fw_r = fw_sb[:].rearrange("p (m i) -> p m i", m=NT)

            def wslot(j):
                s = j % NSLOT
                return wbuf[:, s * WC:(s + 1) * WC]

            def t1slot(j):
                s = j % NT1
                return t1r[:, s * B:(s + 1) * B]

            # sub-bank psum packing for the 9 accumulating U.T tiles: only
            # the first tile in each 2KB bank starts (start zeroes the whole
            # bank's has_written) and only the last stops.
            tile_bytes = B * 4
            bankof = [m * tile_bytes // 2048 for m in range(NT)]
            m_first = [m == 0 or bankof[m] != bankof[m - 1] for m in range(NT)]
            m_last = [m == NT - 1 or bankof[m] != bankof[m + 1] for m in range(NT)]
            hbank = [m * tile_bytes // 2048 for m in range(MT_OWN)]
            h_first = [m == 0 or hbank[m] != hbank[m - 1] for m in range(MT_OWN)]
            h_last = [m == MT_OWN - 1 or hbank[m] != hbank[m + 1]
                      for m in range(MT_OWN)]

            csem = nc.alloc_semaphore("csem")
            dsem = [nc.alloc_semaphore(f"dsem{s}") for s in range(NSLOT)]
            tmm_sem = nc.alloc_semaphore("tmm")
            cp_sem = nc.alloc_semaphore("cp")
            pe_sem = nc.alloc_semaphore("pe")
            sv = nc.alloc_semaphore("sv")     # DVE progress
            sa = nc.alloc_semaphore("sa")     # ACT progress
            pp = nc.alloc_semaphore("pp")     # PE pred progress
            dsm = nc.alloc_semaphore("dsm")   # pred-phase DMA

            with tc.tile_pool(name="pt1", bufs=1, space="PSUM") as pt_pool, \
                 tc.tile_pool(name="pu", bufs=1, space="PSUM") as pu_pool:
                # pt slots are bank-aligned (512 f32 apart) so each T1
                # matmul's start=True only clears its own bank
                pt = pt_pool.tile([128, 1024], F32)
                psum_u = pu_pool.tile([128, NT * B], F32)
                # pred-phase psum aliases dead GEMM psum regions
                psum_w2t = pt[0:MT_OWN, 0:B]
                psum_h = psum_u[0:1, 0:MT_OWN * B]
                psum_p = pt[0:1, 512:512 + B]

                with tc.tile_critical():
                    svc = [0]                 # sv value tracker

                    def dve_inc(inst):
                        inst.then_inc(sv, 1)
                        svc[0] += 1
                        return svc[0]

                    # constants: one DMA, then DVE preps
                    nc.gpsimd.dma_start(cst[:], cst_d[:, :]).then_inc(csem, 16)
                    nc.vector.memset(bias_t[:], -GAIN * SHIFT)
                    msz = nc.vector.memset(zeros64_f[:], 0.0)
                    v_msz = dve_inc(msz)
                    zc = nc.vector.tensor_copy(zeros64_r[:], zeros64_f[:])
                    zc._wait_ge(sv, v_msz)
                    dve_inc(zc)
                    cxr = nc.vector.tensor_copy(xst_r[:], xst)
                    cxr._wait_ge(csem, 16)
                    v_xr = dve_inc(cxr)       # sv: memsets+consts ready

                    def dma_k(j):
                        d = nc.sync.dma_start(
                            wslot(j), wc_d[j * 128:(j + 1) * 128, :])
                        if j >= NSLOT:
                            # slot free once GEMM2(j-NSLOT) fully read it
                            d._wait_ge(pe_sem, j - NSLOT + 1)
                        d.then_inc(dsem[j % NSLOT], 16)

                    def t1_mm(j):
                        mm = nc.tensor.matmul(
                            pt[:, (j % 2) * 512:(j % 2) * 512 + B],
                            wslot(j)[:, MSL:WC], xst_r[:],
                            start=True, stop=True)
                        mm._wait_ge(dsem[j % NSLOT], 16 * (j // NSLOT + 1))
                        mm.then_inc(tmm_sem, 1)

                    def t1_copy(j):
                        cp = nc.vector.tensor_copy(
                            t1slot(j), pt[:, (j % 2) * 512:(j % 2) * 512 + B])
                        cp._wait_ge(tmm_sem, j + 1)
                        cp.then_inc(cp_sem, 1)

                    # PE warmup into pt slot 0: pulls the const/DVE prep
                    # tick into the PE's clock (T1mm(0) overwrites it next)
                    zmm = nc.tensor.matmul(pt[:, 0:B],
                                           zeros64_r[:], xst_r[:],
                                           start=True, stop=True)
                    zmm._wait_ge(sv, v_xr)

                    for j in range(LOOK):
                        dma_k(j)
                    for j in range(2):
                        t1_mm(j)
                        t1_copy(j)

                    for k in range(KT):
                        if k + LOOK < KT:
                            dma_k(k + LOOK)
                        for m in range(NT):
                            mm = nc.tensor.matmul(
                                psum_u[:, m * B:(m + 1) * B],
                                wslot(k)[:, m * 128:(m + 1) * 128],
                                t1slot(k),
                                start=(k == 0 and m_first[m]),
                                stop=(k == KT - 1 and m_last[m]),
                            )
                            if m == 0:
                                mm._wait_ge(cp_sem, k + 1)
                            if m == NT - 1:
                                mm.then_inc(pe_sem, 1)
                        if k + 2 < KT:
                            t1_mm(k + 2)
                            t1_copy(k + 2)

                    cpu = nc.vector.tensor_copy(u_sb[:], psum_u[:])
                    cpu._wait_ge(pe_sem, KT)
                    v_ucp = dve_inc(cpu)

                    # ---- 256-step sigmoid scan ----
                    sa_base = 0
                    for i in range(B):
                        t_t = t_big[:, (i % 2) * NT:(i % 2) * NT + NT]
                        prev = fw0_t if i == 0 else fw_r[:, :, i - 1]
                        add = nc.vector.tensor_add(t_t, prev, u_r[:, :, i])
                        if i > 0:
                            add._wait_ge(sa, i)
                        else:
                            add._wait_ge(sv, v_ucp)
                        v_add = dve_inc(add)
                        act = nc.scalar.activation(
                            fw_r[:, :, i], t_t, AF.Sigmoid,
                            bias=bias_t[:], scale=GAIN)
                        act._wait_ge(sv, v_add)
                        act.then_inc(sa, 1)

                    # ---- partial predictions ----
                    v_prod = []
                    for m in range(MT_OWN):
                        pr = nc.vector.tensor_mul(
                            prod_big[:, m * B:(m + 1) * B],
                            fw_r[:, m, :], xst)
                        if m == 0:
                            pr._wait_ge(sa, B)
                        v_prod.append(dve_inc(pr))
                    sel_mm = nc.tensor.matmul(
                        psum_w2t[:], sel_t, fw_r[0:HID, NT - 1, :],
                        start=True, stop=True)
                    sel_mm._wait_ge(sv, v_prod[0])  # implies sa >= B
                    for m in range(MT_OWN):
                        hm = nc.tensor.matmul(
                            psum_h[0:1, m * B:(m + 1) * B], ones_t,
                            prod_big[:, m * B:(m + 1) * B],
                            start=h_first[m], stop=h_last[m])
                        hm._wait_ge(sv, v_prod[m])
                        if m == MT_OWN - 1:
                            hm.then_inc(pp, 1)
                    cp1 = nc.vector.tensor_copy(w2t_sb[:], psum_w2t[:])
                    cp1._wait_ge(pp, 1)
                    dve_inc(cp1)
                    cp2 = nc.vector.tensor_copy(h_flat[:], psum_h[:])
                    v_hflat = dve_inc(cp2)
                    dh1 = nc.sync.dma_start(hs_d[:, :], h_flat[0:1, :])
                    dh1._wait_ge(sv, v_hflat)
                    dh1.then_inc(dsm, 16)
                    dh2 = nc.sync.dma_start(h_sb[:], hs_d[:, :])
                    dh2._wait_ge(dsm, 16)
                    dh2.then_inc(dsm, 16)
                    rl = nc.vector.tensor_relu(r_sb[:], h_sb[:])
                    rl._wait_ge(dsm, 32)
                    v_relu = dve_inc(rl)
                    pm = nc.vector.tensor_mul(p_sb[:], r_sb[:], w2t_sb[:])
                    pm._wait_ge(sv, v_relu)
                    v_psb = dve_inc(pm)
                    pmm = nc.tensor.matmul(psum_p[:], ones_t[0:MT_OWN, :],
                                           p_sb[:], start=True, stop=True)
                    pmm._wait_ge(sv, v_psb)
                    pmm.then_inc(pp, 1)
                    cp3 = nc.vector.tensor_copy(pred_sb[:], psum_p[:])
                    cp3._wait_ge(pp, 2)
                    v_pred = dve_inc(cp3)
                    dout = nc.sync.dma_start(pred_d[:, :], pred_sb[:])
                    dout._wait_ge(sv, v_pred)
                    dout.then_inc(dsm, 16)
                    if dbg:
                        du = nc.sync.dma_start(u_dbg_d[:, :], u_sb[:])
                        du._wait_ge(sv, v_pred)
                        du.then_inc(dsm, 16)
                        df = nc.sync.dma_start(fw_dbg_d[:, :], fw_sb[:])
                        df._wait_ge(sv, v_pred)
                        df.then_inc(dsm, 16)
                        dt1 = nc.sync.dma_start(
                            t1_dbg_d[:, :], t1r[:].bitcast(F32))
                        dt1._wait_ge(sv, v_pred)
                        dt1.then_inc(dsm, 16)
                        for dd, ss in ((hsb_dbg_d, h_sb), (w2t_dbg_d, w2t_sb),
                                       (hflat_dbg_d, h_flat)):
                            dx = nc.sync.dma_start(dd[:, :], ss[:])
                            dx._wait_ge(sv, v_pred)
                            dx.then_inc(dsm, 16)

    _dedupe_waits(nc)
    return nc


def _dedupe_waits(nc):
    """Collapse duplicate semaphore waits the framework occasionally emits
    (e.g. critical-entry branches) — walrus allows very few sync commands
    per instruction."""
    for fnn in nc.m.functions:
        for blk in fnn.blocks:
            for inst in blk.instructions:
                si = inst.sync_info
                if si is None or not si.on_wait or len(si.on_wait) < 2:
                    continue
                best = {}
                order = []
                for w in si.on_wait:
                    if w.wait_reg is not None or w.wait_mode != "sem-ge-imm":
                        key = ("raw", id(w))
                    else:
                        key = (w.sync_type, w.id, w.wait_mode)
                    if key not in best:
                        best[key] = w
                        order.append(key)
                    elif (w.wait_value or 0) > (best[key].wait_value or 0):
                        best[key] = w
                deduped = [best[k] for k in order]
                if len(deduped) != len(si.on_wait):
                    inst.sync_info = mybir.SyncInfo(
                        on_wait=deduped, on_update=si.on_update)
def _split_noops(nc):
    """Split multi-wait NoOps into single-wait chains (walrus's CTRL_NO
    struct carries very few sync commands). Applied lazily before HW runs
    only — CoreSim rejects instructions without its fake-update records."""
    if getattr(nc, "_noops_split", False):
        return
    nc._noops_split = True
    split_id = [0]
    for fnn in nc.m.functions:
        for blk in fnn.blocks:
            out = []
            changed = False
            for inst in blk.instructions:
                si = inst.sync_info
                if (type(inst).__name__ == "InstNoOp" and si is not None
                        and len(si.on_wait) > 1):
                    changed = True
                    for w in si.on_wait[:-1]:
                        no = mybir.InstNoOp(
                            name=f"noop_waitsplit_{split_id[0]}",
                            text_hint="waitsplit")
                        split_id[0] += 1
                        no.engine = inst.engine
                        no.sync_info = mybir.SyncInfo(
                            on_wait=[w], on_update=[])
                        out.append(no)
                    inst.sync_info = mybir.SyncInfo(
                        on_wait=[si.on_wait[-1]], on_update=si.on_update)
                out.append(inst)
            if changed:
                blk.instructions = out


def _get_nc():
    global _NC_CACHE
    if _NC_CACHE is None:
        _NC_CACHE = _build_bass()
    return _NC_CACHE


def _make_in_maps(x, W1, W2, fw0):
    xs = np.ascontiguousarray(x[:, 0, :].astype(np.float32))       # [256, 128]
    xst = np.ascontiguousarray(xs.T)                                # [128, 256]
    W1 = np.asarray(W1, dtype=np.float32)
    W2 = np.asarray(W2, dtype=np.float32)
    fw0 = np.asarray(fw0, dtype=np.float32)
    ones = np.ones((128, 1), np.float32)

    # k-block-transposed W1: rows k*128+p, col c = W1[k*128+c, p]
    w1bt = np.ascontiguousarray(
        W1.reshape(KT, 128, IN).transpose(0, 2, 1).reshape(K2, 128))

    shared_rows = W2[MT_OWN * 128 * NCORES:, :]                     # [64, 16512]
    fw_shared = np.zeros(128, np.float32)
    fw_shared[0:HID] = fw0[MT_OWN * 128 * NCORES:]

    in_maps = []
    for c in range(NCORES):
        own = W2[c * 1024:(c + 1) * 1024, :]                        # [1024, 16512]
        w2c = np.concatenate([own, shared_rows], axis=0)            # [1088, 16512]
        wcomb = np.concatenate(
            [np.ascontiguousarray(w2c.T), w1bt], axis=1)            # [16512, 1216]
        fw0_t = np.zeros((128, NT), np.float32)
        for m in range(MT_OWN):
            fw0_t[:, m] = fw0[c * 1024 + m * 128: c * 1024 + (m + 1) * 128]
        fw0_t[:, NT - 1] = fw_shared
        sel = np.zeros((HID, MT_OWN), np.float32)
        for m in range(MT_OWN):
            sel[MT_OWN * c + m, m] = 1.0
        cst = np.zeros((128, B + NT + MT_OWN + 1), np.float32)
        cst[:, 0:B] = xst
        cst[:, B:B + NT] = fw0_t
        cst[0:HID, B + NT:B + NT + MT_OWN] = sel
        cst[:, B + NT + MT_OWN] = 1.0
        in_maps.append({
            "wcomb": np.ascontiguousarray(wcomb),
            "cst": cst,
        })
    return in_maps


def kernel(x, W1, W2, fw0, _trace=False, _tmpdir=None):
    nc = _get_nc()
    _split_noops(nc)
    in_maps = _make_in_maps(x, W1, W2, fw0)
    res = run_bass_kernel_spmd(
        nc, in_maps, core_ids=list(range(NCORES)),
        trace=_trace, tmpdir=_tmpdir,
    )
    preds = np.zeros((1, B), np.float64)
    for c in range(NCORES):
        preds += res.results[c]["pred"].astype(np.float64)
    out = preds.astype(np.float32).reshape(B, 1)
    if _trace:
        return out, res
    return out

